# revision 34
# baseline (speedup 1.0000x reference)
"""Trainium2 Bass kernel for SoftMoE (LayerNorm + cosine routing + per-expert MLP).

Sharding: pure data-parallel over batch B=8 -> one batch element per NeuronCore.
No collectives. Each core computes its full (N, D) output slice.

Math notes (per core, x is (N, D)):
  x_ln = LN(x) * gamma + beta
  x_n  = x_ln * t[n],   t[n] = scale / ||x_ln[n]||
  logitsT[es, n] = minv[es] * (mu_raw.T @ x_n.T)   (minv = 1/||mu[:,es]||)
  E = exp(logits)   (cosine logits are bounded, so no max-subtraction needed)
  dispatch = E / sd[es] (col softmax over n); combine = E / sc[n] (row softmax)
  slot_inT = x_n.T @ (E * (1/t)[n])          == x_ln.T @ dispatch_raw
  h  = gelu(sinv_d[es] * (slot_inT.T @ W1) + b1)
  so = h @ W2 + b2
  out[n] = (1/sc[n]) * (E @ so)
sd comes free from the exp eviction's accum_out; sc comes free from an extra
ones-column matmul in the combine accumulation.
"""

import numpy as np
from contextlib import ExitStack

import concourse.bass as bass
import concourse.tile as tile
from concourse import bacc
from concourse import mybir
from concourse.masks import make_identity

FP32 = mybir.dt.float32
BF16 = mybir.dt.bfloat16
AF = mybir.ActivationFunctionType
ALU = mybir.AluOpType
AX = mybir.AxisListType

P = 128
LN_EPS = 1e-5

# CoreSim doesn't implement Gelu; dev_sim flips this to validate the pipeline
# with Tanh standing in for Gelu. Hardware builds keep the real Gelu.
SIM_SAFE_GELU = False


def _bcast_ap(handle, p, free):
    """AP reading a 1-D DRAM tensor broadcast across p partitions."""
    return bass.AP(tensor=handle, offset=0, ap=[[0, p], [1, free]])


def build_softmoe(N, D, E, S, H, *, apply_gamma_beta=True, apply_b1=True,
                  apply_b2=True):
    assert S == P
    ES = E * S
    NT, KD, NE, QH = N // P, D // P, ES // P, H // P
    CN = min(512, N); JN = N // CN       # n-chunks
    CE = min(512, ES); JE = ES // CE     # es-chunks
    CD = min(512, D); JD = D // CD       # d-chunks
    EPC = CE // P                        # experts per es-chunk

    nc = bacc.Bacc(None, target_bir_lowering=False, debug=False)

    x_h = nc.dram_tensor("x", [N, D], FP32, kind="ExternalInput")
    g_h = nc.dram_tensor("gamma", [D], FP32, kind="ExternalInput")
    be_h = nc.dram_tensor("beta", [D], FP32, kind="ExternalInput")
    mu_h = nc.dram_tensor("mu", [D, E, S], FP32, kind="ExternalInput")
    sc_h = nc.dram_tensor("scale", [1], FP32, kind="ExternalInput")
    w1_h = nc.dram_tensor("W1", [E, D, H], FP32, kind="ExternalInput")
    b1_h = nc.dram_tensor("b1", [E, H], FP32, kind="ExternalInput")
    w2_h = nc.dram_tensor("W2", [E, H, D], FP32, kind="ExternalInput")
    b2_h = nc.dram_tensor("b2", [E, D], FP32, kind="ExternalInput")
    out_h = nc.dram_tensor("out", [N, D], FP32, kind="ExternalOutput")

    xn_d = nc.dram_tensor("xn_scr", [N, D], BF16)
    et_d = nc.dram_tensor("et_scr", [ES, N], BF16)

    with tile.TileContext(nc, pool_alloc_mode="queue") as tc, ExitStack() as ctx:
        small = ctx.enter_context(tc.tile_pool(name="small", bufs=1))
        # One PSUM pool for the whole kernel: matmul tiles share the "mmps"
        # tag (6 banks), PE-transpose tiles share "pst" (2 banks). A single
        # live pool avoids cross-phase PSUM-zone reuse deps.
        psum = ctx.enter_context(tc.tile_pool(name="psum", bufs=6, space="PSUM"))

        ones_b = small.tile([P, 1], BF16, tag="ones_b")
        nc.vector.memset(ones_b, 1.0)
        ones_f = small.tile([P, 1], FP32, tag="ones_f")
        nc.vector.memset(ones_f, 1.0)
        s_bc = small.tile([P, 1], FP32, tag="s_bc")
        nc.gpsimd.dma_start(out=s_bc, in_=_bcast_ap(sc_h, P, 1))
        eps_t = small.tile([P, 1], FP32, tag="eps_t")
        nc.vector.memset(eps_t, LN_EPS)
        tinv = small.tile([P, NT], FP32, tag="tinv")
        minv = small.tile([P, NE], FP32, tag="minv")
        sd = small.tile([P, NE], FP32, tag="sd")
        sdinv = small.tile([P, NE], FP32, tag="sdinv")
        ident_b = small.tile([P, P], BF16, tag="ident_b")
        make_identity(nc, ident_b)
        if apply_b1:
            ident_f = small.tile([P, P], FP32, tag="ident_f")
            make_identity(nc, ident_f)
        if apply_b2:
            ones_row = small.tile([1, P], BF16, tag="ones_row")
            nc.vector.memset(ones_row, 1.0)
        if apply_gamma_beta:
            gm_bc = small.tile([P, D], FP32, tag="gm_bc")
            nc.gpsimd.dma_start(out=gm_bc, in_=_bcast_ap(g_h, P, D))
            bt_bc = small.tile([P, D], FP32, tag="bt_bc")
            nc.gpsimd.dma_start(out=bt_bc, in_=_bcast_ap(be_h, P, D))

        # persistent x_n tiles: written by P1, read as dispatch lhsT in P3
        xnkp = ctx.enter_context(tc.tile_pool(name="xnk_pool", bufs=1))
        xnk = [xnkp.tile([P, D], BF16, tag=f"xnk{i}", name=f"xnk{i}")
               for i in range(NT)]

        # ------------- P2a: mu load/cast + column norms (overlaps P1) -------
        mub_ctx = ExitStack()
        mubp = mub_ctx.enter_context(tc.tile_pool(name="mub_pool", bufs=1))
        mub = [mubp.tile([P, ES], BF16, tag=f"mub{k}", name=f"mub{k}")
               for k in range(KD)]

        # ------------- P1: LayerNorm + x_n (bf16, token-major) --------------
        # Fast path (gamma==1, beta==0, scale>0): the LN rstd cancels against
        # the l2 norm: x_n = (x - mean) * c with c = s/sqrt(D*var) and
        # tinv = sqrt(D*var/(var+eps))/s. Only ACT Sqrt is used (a single
        # activation table set; Ln/Exp here would thrash table loads against
        # each other). x_n tiles stay resident in SBUF for the dispatch phase.
        sinv_bc = small.tile([P, 1], FP32, tag="sinv_bc")
        nc.vector.reciprocal(out=sinv_bc[:], in_=s_bc[:])
        with tc.tile_pool(name="p1", bufs=4) as p1, \
                tc.tile_pool(name="p1s", bufs=8) as p1s, \
                tc.tile_pool(name="p2a", bufs=3) as p2a:
            n2 = psum.tile([P, NE], FP32, tag="mmps", name="n2")
            for i in range(max(NT, KD)):
                # interleave mu column loads/norms with x LN tiles so the two
                # DMA streams finish together instead of mu hogging the queues
                if i < KD:
                    k = i
                    nc.gpsimd.dma_start(
                        out=mub[k][:],
                        in_=mu_h[k * P:(k + 1) * P].rearrange("p e s -> p (e s)"))
                    msq = p2a.tile([P, ES], BF16, tag="msq")
                    nc.vector.tensor_mul(msq[:], mub[k][:], mub[k][:])
                    for e in range(NE):
                        nc.tensor.matmul(n2[:, e:e + 1],
                                         msq[:, e * P:(e + 1) * P], ones_b[:],
                                         start=(k == 0), stop=(k == KD - 1),
                                         skip_group_check=True)
                if i >= NT:
                    continue
                xf = p1.tile([P, D], FP32, tag="xf")
                nc.sync.dma_start(out=xf[:], in_=x_h[i * P:(i + 1) * P, :])
                sub = min(512, D)
                nsub = D // sub
                st = p1s.tile([P, nsub, 6], FP32, tag="st")
                for u in range(nsub):
                    nc.vector.bn_stats(out=st[:, u, :],
                                       in_=xf[:, u * sub:(u + 1) * sub])
                mv = p1s.tile([P, 2], FP32, tag="mv")
                nc.vector.bn_aggr(out=mv[:], in_=st[:])
                xnb = xnk[i]
                if not apply_gamma_beta:
                    den = p1s.tile([P, 1], FP32, tag="den")
                    nc.vector.tensor_scalar_add(den[:], mv[:, 1:2], LN_EPS)
                    rden = p1s.tile([P, 1], FP32, tag="rden")
                    nc.vector.reciprocal(out=rden[:], in_=den[:])
                    w_ = p1s.tile([P, 1], FP32, tag="w_")
                    nc.vector.tensor_mul(w_[:], mv[:, 1:2], rden[:])
                    sq1 = p1s.tile([P, 1], FP32, tag="sq1")
                    nc.scalar.activation(out=sq1[:], in_=mv[:, 1:2],
                                         func=AF.Sqrt, scale=float(D))
                    rc = p1s.tile([P, 1], FP32, tag="rc")
                    nc.vector.reciprocal(out=rc[:], in_=sq1[:])
                    c_ = p1s.tile([P, 1], FP32, tag="c_")
                    nc.vector.tensor_scalar_mul(c_[:], rc[:], s_bc[:])
                    sq2 = p1s.tile([P, 1], FP32, tag="sq2")
                    nc.scalar.activation(out=sq2[:], in_=w_[:], func=AF.Sqrt,
                                         scale=float(D))
                    nc.vector.tensor_scalar_mul(tinv[:, i:i + 1], sq2[:],
                                                sinv_bc[:])
                    nc.vector.tensor_scalar(out=xnb[:], in0=xf[:],
                                            scalar1=mv[:, 0:1], scalar2=c_[:],
                                            op0=ALU.subtract, op1=ALU.mult)
                else:
                    lv = p1s.tile([P, 1], FP32, tag="lv")
                    nc.vector.tensor_scalar_add(lv[:], mv[:, 1:2], LN_EPS)
                    q_ = p1s.tile([P, 1], FP32, tag="q_")
                    nc.scalar.activation(out=q_[:], in_=lv[:], func=AF.Sqrt)
                    r = p1s.tile([P, 1], FP32, tag="r")
                    nc.vector.reciprocal(out=r[:], in_=q_[:])
                    xln = p1.tile([P, D], FP32, tag="xln")
                    nc.vector.tensor_scalar(out=xln[:], in0=xf[:],
                                            scalar1=mv[:, 0:1], scalar2=r[:],
                                            op0=ALU.subtract, op1=ALU.mult)
                    nc.vector.tensor_mul(xln[:], xln[:], gm_bc[:])
                    nc.vector.tensor_add(xln[:], xln[:], bt_bc[:])
                    sq = p1.tile([P, D], FP32, tag="sq")
                    nc.vector.tensor_mul(sq[:], xln[:], xln[:])
                    ss = p1s.tile([P, 1], FP32, tag="ss")
                    nc.vector.tensor_reduce(out=ss[:], in_=sq[:], axis=AX.X,
                                            op=ALU.add)
                    qs = p1s.tile([P, 1], FP32, tag="qs")
                    nc.scalar.activation(out=qs[:], in_=ss[:], func=AF.Sqrt)
                    u_ = p1s.tile([P, 1], FP32, tag="u_")
                    nc.vector.reciprocal(out=u_[:], in_=qs[:])
                    t_ = p1s.tile([P, 1], FP32, tag="t_")
                    nc.vector.tensor_scalar_mul(t_[:], u_[:], s_bc[:])
                    nc.vector.reciprocal(out=tinv[:, i:i + 1], in_=t_[:])
                    nc.vector.tensor_scalar_mul(xnb[:], xln[:], t_[:])
                nc.sync.dma_start(out=xn_d[i * P:(i + 1) * P, :], in_=xnb[:])
            sqn = small.tile([P, NE], FP32, tag="sqn")
            nc.scalar.activation(out=sqn[:], in_=n2[:], func=AF.Sqrt)
            nc.vector.reciprocal(out=minv[:], in_=sqn[:])

        # ------------- P2b: logits^T + exp (n-chunk outer, streamed xnT) ----
        # xn_r reloads for the dispatch phase are issued here so the DMAs
        # spread across the logits phase instead of bursting at the boundary.
        sdall = small.tile([P, NE * JN], FP32, tag="sdall")
        with tc.tile_pool(name="xnT_pool", bufs=3) as xntp, \
                tc.tile_pool(name="p2b", bufs=6) as p2b:
            for j in range(JN):
                xntc = xntp.tile([P, KD, CN], BF16, tag="xntc")
                for k in range(KD):
                    eng = nc.sync if k % 2 == 0 else nc.scalar
                    eng.dma_start(
                        out=xntc[:, k, :],
                        in_=xn_d[j * CN:(j + 1) * CN, k * P:(k + 1) * P],
                        transpose=True)
                for e in range(NE):
                    ps = psum.tile([P, CN], FP32, tag="mmps",
                                   name=f"lgps{e}_{j}")
                    for k in range(KD):
                        nc.tensor.matmul(ps[:],
                                         mub[k][:, e * P:(e + 1) * P],
                                         xntc[:, k, :],
                                         start=(k == 0), stop=(k == KD - 1))
                    ett = p2b.tile([P, CN], BF16, tag="ett")
                    nc.scalar.activation(out=ett[:], in_=ps[:], func=AF.Exp,
                                         scale=minv[:, e:e + 1],
                                         accum_out=sdall[:, e * JN + j:
                                                         e * JN + j + 1])
                    nc.sync.dma_start(
                        out=et_d[e * P:(e + 1) * P, j * CN:(j + 1) * CN],
                        in_=ett[:])
            for e in range(NE):
                nc.vector.tensor_reduce(
                    out=sd[:, e:e + 1],
                    in_=sdall[:, e * JN:(e + 1) * JN], axis=AX.X, op=ALU.add)
            nc.vector.reciprocal(out=sdinv[:], in_=sd[:])
        mub_ctx.close()  # release mub pool before the dispatch/MLP phase

        # ------------- P3: dispatch + per-expert MLP (interleaved) ----------
        sop = ctx.enter_context(tc.tile_pool(name="so_pool", bufs=1))
        p3_ctx = ExitStack()
        sitp = p3_ctx.enter_context(tc.tile_pool(name="sit_pool", bufs=1))
        echp = p3_ctx.enter_context(tc.tile_pool(name="ech", bufs=2))
        mlp = p3_ctx.enter_context(tc.tile_pool(name="mlp", bufs=8))
        mlpw2 = p3_ctx.enter_context(tc.tile_pool(name="mlp_w2", bufs=4))
        mlpsm = p3_ctx.enter_context(tc.tile_pool(name="mlp_sm", bufs=4))
        if True:
            xn_r = xnk
            siT = [sitp.tile([P, CE], BF16, tag=f"siT{d}", name=f"siT{d}")
                   for d in range(KD)]
            so = [sop.tile([P, D], BF16, tag=f"so{e}", name=f"so{e}")
                  for e in range(NE)]
            gelu_f = AF.Tanh if SIM_SAFE_GELU else AF.Gelu
            for c in range(JE):
                ech = echp.tile([P, NT, CE], BF16, tag="ech")
                for k in range(NT):
                    eng = nc.sync if k % 2 == 0 else nc.scalar
                    eng.dma_start(
                        out=ech[:, k, :],
                        in_=et_d[c * CE:(c + 1) * CE, k * P:(k + 1) * P],
                        transpose=True)
                    nc.vector.tensor_scalar_mul(ech[:, k, :], ech[:, k, :],
                                                tinv[:, k:k + 1])
                for d in range(KD):
                    ps = psum.tile([P, CE], FP32, tag="mmps", name=f"sips{c}_{d}")
                    for k in range(NT):
                        nc.tensor.matmul(ps[:],
                                         xn_r[k][:, d * P:(d + 1) * P],
                                         ech[:, k, :],
                                         start=(k == 0), stop=(k == NT - 1))
                    nc.vector.tensor_copy(out=siT[d][:], in_=ps[:])
                # MLP for the experts covered by this es-chunk
                for e in range(c * EPC, (c + 1) * EPC):
                    le = e - c * EPC  # expert offset within chunk columns
                    psh = psum.tile([P, H], FP32, tag="mmps", name=f"psh{e}")
                    for k in range(KD):
                        w1b = mlp.tile([P, H], BF16, tag="w1b", bufs=8)
                        nc.gpsimd.dma_start(out=w1b[:],
                                            in_=w1_h[e, k * P:(k + 1) * P, :])
                        nc.tensor.matmul(psh[:],
                                         siT[k][:, le * P:(le + 1) * P],
                                         w1b[:], start=(k == 0),
                                         stop=(k == KD - 1 and not apply_b1))
                    if apply_b1:
                        # psh += outer(sd_e, b1_e); gelu scale then yields
                        # gelu(sdinv*raw + b1)
                        pst0 = psum.tile([P, P], FP32, tag="pst", name=f"psdr{e}", bufs=2)
                        nc.tensor.transpose(pst0[:1, :], sd[:, e:e + 1],
                                            ident_f[:])
                        sdrow = mlpsm.tile([1, P], BF16, tag="sdrow")
                        nc.vector.tensor_copy(out=sdrow[:], in_=pst0[:1, :])
                        b1row = mlpsm.tile([1, H], BF16, tag="b1row")
                        nc.gpsimd.dma_start(out=b1row[:], in_=b1_h[e:e + 1, :])
                        nc.tensor.matmul(psh[:], sdrow[:], b1row[:],
                                         start=False, stop=True)
                    hbf = mlp.tile([P, H], BF16, tag="hbf", bufs=2)
                    nc.scalar.activation(out=hbf[:], in_=psh[:], func=gelu_f,
                                         scale=sdinv[:, e:e + 1])
                    hT = mlp.tile([P, QH, P], BF16, tag="hT", bufs=2)
                    for q in range(QH):
                        pst = psum.tile([P, P], BF16, tag="pst", name=f"pst{e}_{q}", bufs=2)
                        nc.tensor.transpose(pst[:], hbf[:, q * P:(q + 1) * P],
                                            ident_b[:])
                        nc.vector.tensor_copy(out=hT[:, q, :], in_=pst[:])
                    w2q = [mlpw2.tile([P, D], BF16, tag="w2q", bufs=4,
                                      name=f"w2q{e}_{q}") for q in range(QH)]
                    for q in range(QH):
                        nc.gpsimd.dma_start(out=w2q[q][:],
                                            in_=w2_h[e, q * P:(q + 1) * P, :])
                    if apply_b2:
                        b2row = mlpsm.tile([1, D], BF16, tag="b2row")
                        nc.gpsimd.dma_start(out=b2row[:], in_=b2_h[e:e + 1, :])
                    for dch in range(JD):
                        pso = psum.tile([P, CD], FP32, tag="mmps",
                                        name=f"pso{e}_{dch}")
                        for q in range(QH):
                            nc.tensor.matmul(
                                pso[:], hT[:, q, :],
                                w2q[q][:, dch * CD:(dch + 1) * CD],
                                start=(q == 0),
                                stop=(q == QH - 1 and not apply_b2))
                        if apply_b2:
                            nc.tensor.matmul(
                                pso[:], ones_row[:],
                                b2row[:, dch * CD:(dch + 1) * CD],
                                start=False, stop=True)
                        nc.vector.tensor_copy(
                            out=so[e][:, dch * CD:(dch + 1) * CD], in_=pso[:])

            p3_ctx.close()  # release dispatch/MLP pools; keep `so` for P4
            # ------------- P4: combine --------------------------------------
            et_view = et_d[:, :].rearrange("(k p) n -> p k n", p=P)
            with tc.tile_pool(name="p4", bufs=3) as p4, \
                    tc.tile_pool(name="p4s", bufs=4) as p4s:
                for i in range(NT):
                    etb = p4.tile([P, NE, P], BF16, tag="etb")
                    nc.sync.dma_start(out=etb[:],
                                      in_=et_view[:, :, i * P:(i + 1) * P])
                    pso_ = [psum.tile([P, CD], FP32, tag="mmps",
                                      name=f"ops{i}_{j}") for j in range(JD)]
                    pssc = psum.tile([P, 1], FP32, tag="pst", name=f"pssc{i}", bufs=2)
                    for k in range(NE):
                        for dch in range(JD):
                            nc.tensor.matmul(
                                pso_[dch][:], etb[:, k, :],
                                so[k][:, dch * CD:(dch + 1) * CD],
                                start=(k == 0), stop=(k == NE - 1))
                        nc.tensor.matmul(pssc[:], etb[:, k, :], ones_b[:],
                                         start=(k == 0), stop=(k == NE - 1))
                    scinv = p4s.tile([P, 1], FP32, tag="scinv")
                    nc.vector.reciprocal(out=scinv[:], in_=pssc[:])
                    outt = p4.tile([P, D], FP32, tag="outt")
                    for dch in range(JD):
                        nc.scalar.activation(
                            out=outt[:, dch * CD:(dch + 1) * CD],
                            in_=pso_[dch][:], func=AF.Copy, scale=scinv[:])
                    nc.sync.dma_start(out=out_h[i * P:(i + 1) * P, :],
                                      in_=outt[:])
    nc.compile()
    return nc


_NC_CACHE = {}


def _get_nc(N, D, E, S, H, flags):
    key = (N, D, E, S, H, flags)
    if key not in _NC_CACHE:
        _NC_CACHE[key] = build_softmoe(
            N, D, E, S, H, apply_gamma_beta=flags[0], apply_b1=flags[1],
            apply_b2=flags[2])
    return _NC_CACHE[key]


def kernel(x, gamma, beta, mu, scale, W1, b1, W2, b2):
    from concourse.bass_utils import run_bass_kernel_spmd

    x = np.ascontiguousarray(np.asarray(x, dtype=np.float32))
    gamma = np.ascontiguousarray(np.asarray(gamma, dtype=np.float32))
    beta = np.ascontiguousarray(np.asarray(beta, dtype=np.float32))
    mu = np.ascontiguousarray(np.asarray(mu, dtype=np.float32))
    scale = np.ascontiguousarray(np.asarray(scale, dtype=np.float32))
    W1 = np.ascontiguousarray(np.asarray(W1, dtype=np.float32))
    b1 = np.ascontiguousarray(np.asarray(b1, dtype=np.float32))
    W2 = np.ascontiguousarray(np.asarray(W2, dtype=np.float32))
    b2 = np.ascontiguousarray(np.asarray(b2, dtype=np.float32))

    B, N, D = x.shape
    _, E, S = mu.shape
    H = W1.shape[2]
    n_cores = 8
    assert B == n_cores, f"kernel hardcoded for B == {n_cores}, got {B}"

    flags = (
        # generic LN path also needed when scale <= 0 (fast path takes ln(s))
        bool(np.any(gamma != 1.0) or np.any(beta != 0.0)
             or np.any(scale <= 0.0)),
        bool(np.any(b1 != 0.0)),
        bool(np.any(b2 != 0.0)),
    )
    nc = _get_nc(N, D, E, S, H, flags)

    shared = dict(gamma=gamma, beta=beta, mu=mu, scale=scale, W1=W1, b1=b1,
                  W2=W2, b2=b2)
    in_maps = [dict(x=x[b], **shared) for b in range(n_cores)]
    import os
    trace = bool(os.environ.get("SOFTMOE_TRACE"))
    res = run_bass_kernel_spmd(nc, in_maps, core_ids=list(range(n_cores)),
                               trace=trace)
    global LAST_RESULT
    LAST_RESULT = res
    return np.stack([r["out"] for r in res.results], axis=0)


LAST_RESULT = None
